# revision 31
# baseline (speedup 1.0000x reference)
"""Differential attention (B=2, S=2048, HS=1024, H=16, KV=4, D=64) on 8 trn2 cores.

Sharding: core c = (b, g) with b = c // 4 (data parallel on batch) and
g = c % 4 (tensor parallel over the 4 KV head groups; each core owns the
4 query heads of its group).  Each core computes its 4 heads' normed
attention output and a row-parallel partial of the output projection
(out_pt = (O_heads @ Wo_rows)^T, bf16); the host upcasts and sums the 4
partials per batch.

All matmul operands are bf16 (PSUM accumulation stays fp32): 1 PE
cycle/row with no small-tile penalty, half the DMA traffic, and 2x DVE
throughput on the element-wise tail.  Inputs are loaded with a handful of
wide batched DMAs (HWDGE issues serialize at ~630ns), xt first so the
first projection can start immediately.

proj(qt) is emitted as a generator of small PE units (K/Q/V-chunk
matmuls + rope), interleaved into attention so the in-order PE queue
always has independent ready work wherever the S->exp->U chain stalls.
RoPE: rot(Q)^T is a cheap 128-contraction matmul against a signed
permutation matrix (rot(q)[d<32] = -q[d+32], rot(q)[d>=32] = +q[d-32]
per 64-block), then q*cos + rot(q)*sin on DVE.  V^T is computed directly
per 128-wide k tile (lhsT = x^T chunk, rhs = Wv chunk): no transposes.

attention(qt), per head: flash-style causal attention over k tiles,
S^T[k,q] strips via two 64-contraction matmuls, P = exp(S/8) on ACT
(no row-max: scores are O(5); diagonal strips exp the two halves
separately to skip the dead zone), STAGE=8 k-tiles of S/exp emitted
ahead of the U matmuls, causal wedge zeroed by one dual-block
mask-multiply on DVE (stride-0 broadcast of a triangular mask),
U^T[128,q] += [V|ones].T @ P — the ones block replicates the softmax
denominator onto partitions 64..127.  The epilogue reads across
partition bases (legal for DVE), so no SBUF shifts are needed:
wri = 1/psu[64:128], O = U1*wri1 - lam*U2*wri2 (lam folded into V2),
plus this head's O^2 row-sum whose PE matmul is deferred to the next
head's prefetch to avoid blocking the queue.

rms(qt): fused ssq row gets one Ln + one Exp on ACT (both live in the
same activation table set as the softmax Exp — a preloaded
InstLoadActFuncSet keeps walrus from thrashing tables), then
full-tile partition_broadcasts (Pool) and one multiply per head pair.
For the last q-tile the rms is split per head-pair to shorten the tail.

wo(qt): partial^T = Wo_rows.T @ O_norm^T; 4 oc blocks share one SBUF
staging tile and one batched store DMA.

Pipeline: proj(qt+1) units fill attention(qt); rms(qt-1) at head slot 1,
wo(qt-1) halves at head slots 2/3.  PSUM: psS pairs [128,1024]
double-buffered (4 banks) + psU [128,1024] single (2) + aux ring (2).
"""

import math
import sys

import numpy as np

try:
    import concourse.bass as bass  # noqa: F401
except ImportError:
    sys.path.insert(0, "/opt/trn_rl_repo")

import concourse.bass as bass
import concourse.tile as tile
from concourse import bacc, mybir
from concourse import bass_utils

f32 = mybir.dt.float32
bf16 = mybir.dt.bfloat16
AF = mybir.ActivationFunctionType
ALU = mybir.AluOpType

B, S, HS = 2, 2048, 1024
H, KV, D = 16, 4, 64
NHL = 4            # query heads per core
NQT = 4            # q tiles of 512
QTW = 512
NKT = 16           # k tiles of 128
NHS = 8            # hs tiles of 128
NEG = -1e9
EPS = 1e-5

_prog_cache = {}
PHASE_LOG = []


def _build_program(lam: float):
    nc = bacc.Bacc("TRN2", target_bir_lowering=False, debug=False,
                   enable_asserts=False, num_devices=8)
    PHASE_LOG.clear()

    def mark(label):
        PHASE_LOG.append((label, nc.next_id()))

    xt = nc.dram_tensor("xt", [HS, S], bf16, kind="ExternalInput").ap()
    wq = nc.dram_tensor("wq", [HS, 512], bf16, kind="ExternalInput").ap()
    wk = nc.dram_tensor("wk", [HS, 128], bf16, kind="ExternalInput").ap()
    wv = nc.dram_tensor("wv", [HS, 64], bf16, kind="ExternalInput").ap()
    wo = nc.dram_tensor("wo", [256, HS], bf16, kind="ExternalInput").ap()
    perm = nc.dram_tensor("perm", [128, 128], bf16, kind="ExternalInput").ap()
    cos_t = nc.dram_tensor("cos_t", [128, S], bf16, kind="ExternalInput").ap()
    sin_t = nc.dram_tensor("sin_t", [128, S], bf16, kind="ExternalInput").ap()
    trimask = nc.dram_tensor("trimask", [128, 128], bf16,
                             kind="ExternalInput").ap()
    out_pt = nc.dram_tensor("out_pt", [HS, S], bf16, kind="ExternalOutput").ap()
    dbg = {}
    if DEBUG_DUMP:
        for nm, shp in (("dbg_k", [128, S]), ("dbg_q00", [128, 512]),
                        ("dbg_va0", [128, 128]), ("dbg_vb0", [128, 128]),
                        ("dbg_op0", [128, 512]), ("dbg_on0", [128, 512]),
                        ("dbg_ssqr0", [1, 2048]), ("dbg_p", [128, 1024])):
            dbg[nm] = nc.dram_tensor(nm, shp, f32, kind="ExternalOutput").ap()

    with tile.TileContext(nc) as tc:
        with tc.tile_pool(name="persist", bufs=1) as pp, \
             tc.tile_pool(name="loc", bufs=2) as loc, \
             tc.tile_pool(name="pwk", bufs=2) as pwk, \
             tc.tile_pool(name="patt", bufs=18) as pa, \
             tc.tile_pool(name="ep", bufs=2) as pe, \
             tc.tile_pool(name="rmsp", bufs=2) as prm, \
             tc.psum_pool(name="ps", bufs=2) as ps_:

            # preload the act-func set that holds BOTH Exp and Ln so the
            # table-load pass never has to switch sets mid-stream
            from concourse.hw_specs import get_activation_tables
            _tables = list(get_activation_tables(nc.m.arch).items())
            _set_id = next(i for i, (_, fs) in enumerate(_tables)
                           if AF.Exp in fs and AF.Ln in fs)
            _ld = mybir.InstLoadActFuncSet(
                name=nc.get_next_instruction_name(),
                act_func_set_id=_set_id, ins=[], outs=[])
            nc.scalar.add_instruction(_ld)

            # Batched loads: one wide tile per weight, few DMA issues
            # (HWDGE serializes issues at ~630ns each). xt0 is issued first
            # by the main loop; K-projection weights feed the first matmuls.
            wk_a = pp.tile([128, 8 * 128], bf16, name="wk", tag="wk")
            nc.sync.dma_start(
                wk_a[:].rearrange("p (h c) -> p h c", h=NHS),
                wk[:].rearrange("(h p) c -> p h c", h=NHS))
            cos_sb = pp.tile([128, S], bf16, name="cos", tag="cos")
            nc.scalar.dma_start(cos_sb[:], cos_t[:])
            sin_sb = pp.tile([128, S], bf16, name="sin", tag="sin")
            nc.sync.dma_start(sin_sb[:], sin_t[:])
            perm_sb = pp.tile([128, 128], bf16, name="perm", tag="perm")
            nc.scalar.dma_start(perm_sb[:], perm[:])
            wq_a = pp.tile([128, 8 * 512], bf16, name="wq", tag="wq")
            for t in range(2):
                nc.sync.dma_start(
                    wq_a[:, t * 2048:(t + 1) * 2048].rearrange(
                        "p (h c) -> p h c", h=4),
                    wq[t * 512:(t + 1) * 512, :].rearrange(
                        "(h p) c -> p h c", h=4))
            wv_a = pp.tile([128, 8 * 64], bf16, name="wv", tag="wv")
            nc.scalar.dma_start(
                wv_a[:].rearrange("p (h c) -> p h c", h=NHS),
                wv[:].rearrange("(h p) c -> p h c", h=NHS))
            ones_sb = pp.tile([128, 64], bf16, name="ones", tag="ones")
            nc.vector.memset(ones_sb[:], 1.0)
            tri_sb = pp.tile([128, 128], bf16, name="tri", tag="tri")
            nc.sync.dma_start(tri_sb[:], trimask[:])
            wo_sb = []
            for t in range(2):
                w = pp.tile([128, HS], bf16, name=f"wo{t}", tag=f"wo{t}")
                nc.scalar.dma_start(w[:], wo[t * 128:(t + 1) * 128, :])
                wo_sb.append(w)
            k_sb = pp.tile([128, S], bf16, name="k", tag="k")
            va = [pp.tile([128, 128], bf16, name=f"va{kt}", tag=f"va{kt}")
                  for kt in range(NKT)]
            vb = [pp.tile([128, 128], bf16, name=f"vb{kt}", tag=f"vb{kt}")
                  for kt in range(NKT)]
            for kt in range(NKT):
                nc.vector.memset(va[kt][:, 64:128], 1.0)
                nc.vector.memset(vb[kt][:, 64:128], 1.0)
            eps_sb = pp.tile([128, 1], f32, name="eps", tag="eps")
            nc.vector.memset(eps_sb[:], EPS)

            def rope_block(ps, dst, qlo, qhi, dst_sb=None):
                # dst = q*cos + rot(q)*sin; rot via perm matmul on PE
                q_sb = dst_sb
                if q_sb is None:
                    q_sb = pwk.tile([128, QTW], bf16, name="qsb", tag="qsb")
                nc.vector.tensor_copy(q_sb[:], ps[:])
                psr = ps_.tile([128, QTW], f32, name="psr", tag="aux")
                nc.tensor.matmul(psr[:], perm_sb[:], q_sb[:],
                                 start=True, stop=True)
                qc = pwk.tile([128, QTW], bf16, name="qc", tag="qc")
                nc.vector.tensor_mul(qc[:], q_sb[:], cos_sb[:, qlo:qhi])
                qs = pwk.tile([128, QTW], bf16, name="qs", tag="qs")
                nc.vector.tensor_mul(qs[:], psr[:], sin_sb[:, qlo:qhi])
                nc.vector.tensor_add(dst, qc[:], qs[:])

            def emit_xt_loads(qt, state):
                mark(f"xt{qt}")
                qlo, qhi = qt * QTW, (qt + 1) * QTW
                xt_a = pwk.tile([128, 8 * QTW], bf16, name="xt", tag="xt")
                for t in range(2):
                    eng = nc.scalar if t == 0 else nc.sync
                    eng.dma_start(
                        xt_a[:, t * 2048:(t + 1) * 2048].rearrange(
                            "p (h c) -> p h c", h=4),
                        xt[t * 512:(t + 1) * 512, qlo:qhi].rearrange(
                            "(h p) c -> p h c", h=4))
                state[(qt, "xt")] = xt_a

            def emit_proj_piece(qt, piece, state):
                mark(f"proj{qt}.p{piece}")
                qlo, qhi = qt * QTW, (qt + 1) * QTW
                xt_a = state[(qt, "xt")]
                if piece == 0:
                    psk = ps_.tile([128, QTW], f32, name="psk", tag="aux")
                    for hs in range(NHS):
                        nc.tensor.matmul(
                            psk[:], wk_a[:, hs * 128:(hs + 1) * 128],
                            xt_a[:, hs * 512:(hs + 1) * 512],
                            start=(hs == 0), stop=(hs == NHS - 1))
                    rope_block(psk, k_sb[:, qlo:qhi], qlo, qhi)
                elif piece <= 4:
                    j = piece - 1
                    if j == 0:
                        state[qt] = [loc.tile([128, QTW], bf16, name=f"q{jj}",
                                              tag=f"q{jj}")
                                     for jj in range(NHL)]
                    qloc = state[qt]
                    psq = ps_.tile([128, QTW], f32, name="psq", tag="aux")
                    for hs in range(NHS):
                        nc.tensor.matmul(
                            psq[:],
                            wq_a[:, hs * 512 + j * 128:hs * 512 + (j + 1) * 128],
                            xt_a[:, hs * 512:(hs + 1) * 512],
                            start=(hs == 0), stop=(hs == NHS - 1))
                    rope_block(psq, qloc[j][:], qlo, qhi)
                else:
                    for kk in range(4):
                        kt = 4 * qt + kk
                        psvt = ps_.tile([128, 64], f32, name="psvt", tag="aux")
                        for hs in range(NHS):
                            nc.tensor.matmul(
                                psvt[:],
                                xt_a[:, hs * 512 + kk * 128:
                                     hs * 512 + (kk + 1) * 128],
                                wv_a[:, hs * 64:(hs + 1) * 64],
                                start=(hs == 0), stop=(hs == NHS - 1))
                        nc.vector.tensor_copy(va[kt][:, 0:64], psvt[:])
                        nc.vector.tensor_scalar_mul(vb[kt][:, 0:64], psvt[:],
                                                    lam)

            def emit_att_head(qt, j, state):
                mark(f"att{qt}.{j}")
                qloc = state[qt]
                if DEBUG_DUMP and qt == NQT - 1 and j == 0:
                    dk = pe.tile([128, S], f32, name="dk", tag="dk")
                    nc.vector.tensor_copy(dk[:], k_sb[:])
                    nc.sync.dma_start(dbg["dbg_k"][:], dk[:])
                if DEBUG_DUMP and qt == 0 and j == 0:
                    for nm, t_ in (("dbg_q00", qloc[0]), ("dbg_va0", va[0]),
                                   ("dbg_vb0", vb[0])):
                        p_, f_ = t_[:].shape
                        d_sb = pe.tile([128, 512], f32, name=f"e{nm}",
                                       tag="dbgd", bufs=4)
                        nc.vector.tensor_copy(d_sb[0:p_, 0:f_], t_[:])
                        nc.sync.dma_start(dbg[nm][:], d_sb[0:p_, 0:f_])
                if j == 0:
                    state[(qt, "op")] = [loc.tile([128, QTW], bf16,
                                                  name=f"op{t}", tag=f"op{t}")
                                         for t in range(2)]
                    state[(qt, "on")] = [loc.tile([128, QTW], bf16,
                                                  name=f"on{t}", tag=f"on{t}")
                                         for t in range(2)]
                opair = state[(qt, "op")]
                half, pt = (j % 2) * 64, j // 2
                last_kt = 4 * qt + 3
                psu = ps_.tile([128, 2 * QTW], f32, name="psu", tag="psU",
                               bufs=1)
                p12s = {}

                def emit_s_exp(kt):
                    jd = kt - 4 * qt
                    q0 = 128 * jd if jd >= 0 else 0
                    pss = ps_.tile([128, 2 * QTW], f32, name="pss", tag="psS")
                    nc.tensor.matmul(
                        pss[:, q0:QTW],
                        k_sb[0:64, kt * 128:(kt + 1) * 128],
                        qloc[j][0:64, q0:QTW],
                        start=True, stop=True, skip_group_check=True)
                    nc.tensor.matmul(
                        pss[:, QTW + q0:2 * QTW],
                        k_sb[64:128, kt * 128:(kt + 1) * 128],
                        qloc[j][64:128, q0:QTW],
                        start=True, stop=True, skip_group_check=True)
                    p12 = pa.tile([128, 2 * QTW], bf16, name="p12", tag="p12")
                    if q0 == 0:
                        nc.scalar.activation(p12[:, 0:2 * QTW], pss[:, 0:2 * QTW],
                                             AF.Exp, scale=0.125)
                    else:
                        nc.scalar.activation(p12[:, q0:QTW], pss[:, q0:QTW],
                                             AF.Exp, scale=0.125)
                        nc.scalar.activation(p12[:, QTW + q0:2 * QTW],
                                             pss[:, QTW + q0:2 * QTW],
                                             AF.Exp, scale=0.125)
                    if jd >= 0:
                        wap = p12[:].rearrange("p (b q) -> p b q",
                                               b=2)[:, :, q0:q0 + 128]
                        msk = tri_sb[:].unsqueeze(1).broadcast_to(
                            [128, 2, 128])
                        nc.vector.tensor_mul(wap, wap, msk)
                    p12s[kt] = p12

                STAGE = 8
                for kt in range(min(STAGE, last_kt + 1)):
                    emit_s_exp(kt)
                for kt in range(last_kt + 1):
                    if kt + STAGE <= last_kt:
                        emit_s_exp(kt + STAGE)
                    jd = kt - 4 * qt
                    q0 = 128 * jd if jd >= 0 else 0
                    p12 = p12s.pop(kt)
                    nc.tensor.matmul(
                        psu[:, q0:QTW], va[kt][:], p12[:, q0:QTW],
                        start=(kt == 0), stop=(kt == last_kt),
                        skip_group_check=True)
                    nc.tensor.matmul(
                        psu[:, QTW + q0:2 * QTW], vb[kt][:],
                        p12[:, QTW + q0:2 * QTW],
                        start=(kt == 0), stop=(kt == last_kt),
                        skip_group_check=True)
                    if filler is not None and kt == last_kt // 2:
                        next(filler, None)
                if filler is not None:
                    for _ in range(4):
                        next(filler, None)
                mark(f"epi{qt}.{j}")
                # epilogue: O^T = U1/r1 - lam*U2/r2  (no PE ops, no shifts:
                # operands read across partition bases)
                wri = pe.tile([64, 2 * QTW], f32, name="wri", tag="wri")
                nc.vector.reciprocal(wri[:], psu[64:128, :])
                t1 = pe.tile([64, QTW], bf16, name="t1", tag="t1")
                nc.vector.tensor_mul(t1[:], psu[0:64, 0:QTW], wri[0:64, 0:QTW])
                t2 = pe.tile([64, QTW], bf16, name="t2", tag="t2")
                nc.vector.tensor_mul(t2[:], psu[0:64, QTW:2 * QTW],
                                     wri[0:64, QTW:2 * QTW])
                nc.gpsimd.tensor_sub(opair[pt][half:half + 64, :], t1[:], t2[:])
                # fused ssq contribution for the rms: row-sum of O^2
                if j == 0:
                    state[(qt, "ssqr")] = prm.tile([1, 4 * QTW], f32,
                                                   name="ssqr", tag="ssqr")
                ssqr = state[(qt, "ssqr")]
                osq = prm.tile([64, QTW], bf16, name="osq", tag="osq")
                nc.vector.tensor_mul(osq[:], opair[pt][half:half + 64, :],
                                     opair[pt][half:half + 64, :])
                psss = ps_.tile([1, QTW], f32, name="psss", tag="aux")
                nc.tensor.matmul(psss[:], ones_sb[0:64, 0:1], osq[:],
                                 start=True, stop=True, skip_group_check=True)
                nc.vector.tensor_copy(ssqr[0:1, j * QTW:(j + 1) * QTW], psss[:])

            def emit_rms(qt, state, pts=(0, 1)):
                mark(f"rms{qt}")
                opair = state[(qt, "op")]
                onq = state[(qt, "on")]
                ssqr = state[(qt, "ssqr")]
                lo, hi = 2 * QTW * pts[0], 2 * QTW * (pts[-1] + 1)
                # rms factor = exp(-0.5*ln(ssq/64+eps)); Ln/Exp share the
                # softmax act table (no table reloads)
                sqr = prm.tile([1, 4 * QTW], f32, name="sqr", tag="sqr")
                nc.scalar.activation(sqr[0:1, lo:hi], ssqr[0:1, lo:hi],
                                     AF.Ln, scale=1.0 / 64.0,
                                     bias=eps_sb[0:1, 0:1])
                rmq = prm.tile([1, 4 * QTW], bf16, name="rmq", tag="rmq")
                nc.scalar.activation(rmq[0:1, lo:hi], sqr[0:1, lo:hi],
                                     AF.Exp, scale=-0.5)
                for pt in pts:
                    rsa = prm.tile([128, QTW], bf16, name="rsa", tag="rsa",
                                   bufs=2)
                    nc.gpsimd.partition_broadcast(
                        rsa[:], rmq[0:1, 2 * pt * QTW:(2 * pt + 1) * QTW])
                    rsb = prm.tile([128, QTW], bf16, name="rsb", tag="rsb",
                                   bufs=2)
                    nc.gpsimd.partition_broadcast(
                        rsb[:], rmq[0:1, (2 * pt + 1) * QTW:(2 * pt + 2) * QTW])
                    nc.vector.tensor_mul(onq[pt][0:64, :], opair[pt][0:64, :],
                                         rsa[0:64, :])
                    nc.vector.tensor_mul(onq[pt][64:128, :],
                                         opair[pt][64:128, :], rsb[64:128, :])
                if DEBUG_DUMP and qt == 0:
                    for nm, t_ in (("dbg_op0", opair[0]), ("dbg_on0", onq[0])):
                        d_sb = pe.tile([128, 512], f32, name=f"d{nm}", tag="dbgd",
                                       bufs=4)
                        nc.vector.tensor_copy(d_sb[:], t_[:])
                        nc.sync.dma_start(dbg[nm][:], d_sb[:])
                    dssq = pe.tile([1, 2048], f32, name="dssq", tag="dssq")
                    nc.vector.tensor_copy(dssq[:], ssqr[:])
                    nc.sync.dma_start(dbg["dbg_ssqr0"][:], dssq[:])

            def emit_wo(qt, state, half):
                mark(f"wo{qt}.{half}")
                qlo, qhi = qt * QTW, (qt + 1) * QTW
                onq = state[(qt, "on")]
                for pair in range(2):
                    oc0 = 4 * half + 2 * pair
                    ow = prm.tile([128, 2 * QTW], bf16, name="ow", tag="ow")
                    for i, oc in enumerate((oc0, oc0 + 1)):
                        psw = ps_.tile([128, QTW], f32, name="psw", tag="aux")
                        nc.tensor.matmul(
                            psw[:], wo_sb[0][:, oc * 128:(oc + 1) * 128],
                            onq[0][:], start=True, stop=False)
                        nc.tensor.matmul(
                            psw[:], wo_sb[1][:, oc * 128:(oc + 1) * 128],
                            onq[1][:], start=False, stop=True)
                        nc.vector.tensor_copy(ow[:, i * QTW:(i + 1) * QTW],
                                               psw[:])
                    nc.sync.dma_start(
                        out_pt[oc0 * 128:(oc0 + 2) * 128,
                               qlo:qhi].rearrange("(b p) c -> p b c", b=2),
                        ow[:].rearrange("p (b c) -> p b c", b=2))

            state = {}
            emit_xt_loads(0, state)
            for piece in range(6):
                emit_proj_piece(0, piece, state)
            for qt in range(NQT):
                if qt < NQT - 1:
                    emit_xt_loads(qt + 1, state)
                for j in range(NHL):
                    emit_att_head(qt, j, state)
                    if qt < NQT - 1:
                        for piece in ((0, 1), (2,), (3,), (4, 5))[j]:
                            emit_proj_piece(qt + 1, piece, state)
                    if j == 1 and qt > 0:
                        emit_rms(qt - 1, state)
                    if j == 1 and qt == NQT - 1:
                        emit_rms(qt, state, pts=(0,))
                    if j == 2 and qt > 0:
                        emit_wo(qt - 1, state, half=0)
                    if j == 3 and qt > 0:
                        emit_wo(qt - 1, state, half=1)
            emit_rms(NQT - 1, state, pts=(1,))
            emit_wo(NQT - 1, state, half=0)
            emit_wo(NQT - 1, state, half=1)
    nc.compile()
    return nc


def get_program(lam: float):
    key = round(float(lam), 9)
    if key not in _prog_cache:
        _prog_cache[key] = _build_program(float(lam))
    return _prog_cache[key]


def _bf16():
    import ml_dtypes
    return ml_dtypes.bfloat16


def _perm_mat():
    # psr = perm.T @ q : psr[p] = -q[p+32] for p%64<32, +q[p-32] for p%64>=32
    p = np.zeros((128, 128), np.float32)
    for o in range(128):
        if o % 64 < 32:
            p[o + 32, o] = -1.0
        else:
            p[o - 32, o] = 1.0
    return p


def _host_inputs(x, rope_cos, rope_sin, Wq, Wk, Wv, Wo, subln_w, lam):
    bf = _bf16()
    cos_t = np.ascontiguousarray(np.tile(rope_cos.T, (4, 1))).astype(bf)
    sin_t = np.ascontiguousarray(np.tile(rope_sin.T, (4, 1))).astype(bf)
    perm = _perm_mat().astype(bf)
    tri = np.triu(np.ones((128, 128), np.float32)).astype(bf)
    sub4 = np.tile(subln_w.astype(np.float32), 4)[:, None]

    in_maps = []
    for c in range(8):
        b, g = c // 4, c % 4
        xtc = np.ascontiguousarray(x[b].T).astype(bf)
        cols = []
        for j in range(NHL):
            h = 4 * g + j
            cols.append(Wq[:, h * 64:(h + 1) * 64])
            cols.append(Wq[:, (H + h) * 64:(H + h + 1) * 64])
        wq_c = np.ascontiguousarray(np.concatenate(cols, axis=1)).astype(bf)
        wk_c = np.ascontiguousarray(np.concatenate(
            [Wk[:, g * 64:(g + 1) * 64], Wk[:, (KV + g) * 64:(KV + g + 1) * 64]],
            axis=1)).astype(bf)
        wv_c = np.ascontiguousarray(Wv[:, g * 64:(g + 1) * 64]).astype(bf)
        wo_c = np.ascontiguousarray(
            Wo[g * 256:(g + 1) * 256, :] * sub4).astype(bf)
        in_maps.append({
            "xt": xtc, "wq": wq_c, "wk": wk_c, "wv": wv_c, "wo": wo_c,
            "perm": perm, "cos_t": cos_t, "sin_t": sin_t, "trimask": tri,
        })
    return in_maps


def _compute_lam(lambda_q1, lambda_k1, lambda_q2, lambda_k2):
    li = 0.8 - 0.6 * math.exp(-0.3)
    l1 = np.exp(np.dot(lambda_q1.astype(np.float32), lambda_k1.astype(np.float32)))
    l2 = np.exp(np.dot(lambda_q2.astype(np.float32), lambda_k2.astype(np.float32)))
    return float(l1 - l2 + li)


def _numpy_reference(x, rope_cos, rope_sin, attention_mask, Wq, Wk, Wv, Wo,
                     lambda_q1, lambda_k1, lambda_q2, lambda_k2, subln_w):
    """Pure-numpy fallback, only used if the mask is not the expected causal one."""
    bsz, seq_len, _ = x.shape

    def rope(t):
        c = np.concatenate([rope_cos, rope_cos], axis=-1)[None, None]
        s = np.concatenate([rope_sin, rope_sin], axis=-1)[None, None]
        t1, t2 = np.split(t, 2, axis=-1)
        rot = np.concatenate([-t2, t1], axis=-1)
        return t * c + rot * s

    q = (x @ Wq).reshape(bsz, seq_len, 2 * H, D)
    q1 = np.transpose(q[:, :, :H], (0, 2, 1, 3))
    q2 = np.transpose(q[:, :, H:], (0, 2, 1, 3))
    k = (x @ Wk).reshape(bsz, seq_len, 2 * KV, D)
    k1 = np.transpose(k[:, :, :KV], (0, 2, 1, 3))
    k2 = np.transpose(k[:, :, KV:], (0, 2, 1, 3))
    v = np.transpose((x @ Wv).reshape(bsz, seq_len, KV, D), (0, 2, 1, 3))
    q1, q2, k1, k2 = rope(q1), rope(q2), rope(k1), rope(k2)
    gr = H // KV
    k1 = np.repeat(k1, gr, axis=1)
    k2 = np.repeat(k2, gr, axis=1)
    v = np.repeat(v, gr, axis=1)
    scale = 1.0 / math.sqrt(D)

    def smax(a):
        a = a - a.max(axis=-1, keepdims=True)
        e = np.exp(a)
        return e / e.sum(axis=-1, keepdims=True)

    a1 = smax(np.einsum("bhqd,bhkd->bhqk", q1, k1) * scale + attention_mask)
    a2 = smax(np.einsum("bhqd,bhkd->bhqk", q2, k2) * scale + attention_mask)
    lam = _compute_lam(lambda_q1, lambda_k1, lambda_q2, lambda_k2)
    attn = a1 - lam * a2
    out = np.einsum("bhqk,bhkd->bhqd", attn, v)
    inv = 1.0 / np.sqrt(np.mean(out * out, axis=-1, keepdims=True) + EPS)
    out = out * inv * subln_w
    out = np.transpose(out, (0, 2, 1, 3)).reshape(bsz, seq_len, HS)
    return (out @ Wo).astype(np.float32)


LAST_RESULT = None


def kernel(x, rope_cos, rope_sin, attention_mask, Wq, Wk, Wv, Wo,
           lambda_q1, lambda_k1, lambda_q2, lambda_k2, subln_w):
    global LAST_RESULT
    x = np.asarray(x, np.float32)
    kk, qq = np.arange(S)[:, None], np.arange(S)[None, :]
    causal = np.where(qq <= kk, 0.0, NEG).astype(np.float32)[None, None]
    am = np.asarray(attention_mask, np.float32)
    if am.shape != (1, 1, S, S) or not np.array_equal(am, causal):
        return _numpy_reference(x, rope_cos, rope_sin, am, Wq, Wk, Wv, Wo,
                                lambda_q1, lambda_k1, lambda_q2, lambda_k2,
                                subln_w)

    lam = _compute_lam(lambda_q1, lambda_k1, lambda_q2, lambda_k2)
    nc = get_program(lam)
    in_maps = _host_inputs(x, np.asarray(rope_cos, np.float32),
                           np.asarray(rope_sin, np.float32),
                           np.asarray(Wq, np.float32), np.asarray(Wk, np.float32),
                           np.asarray(Wv, np.float32), np.asarray(Wo, np.float32),
                           np.asarray(subln_w, np.float32), lam)
    res = bass_utils.run_bass_kernel_spmd(nc, in_maps, core_ids=list(range(8)))
    LAST_RESULT = res
    y = np.zeros((B, S, HS), np.float32)
    for c in range(8):
        y[c // 4] += res.results[c]["out_pt"].T.astype(np.float32)
    return y
